# revision 1
# baseline (speedup 1.0000x reference)
"""Causal self-attention with dense global prefix, tensor-parallel over heads
across 8 Trainium2 NeuronCores.

Reference computation (T=4096, C=1024, H=16, D=64):
    qkv = x @ w_attn; q,k,v per head; scores = q k^T / sqrt(D)
    mask = causal | (col < num_frames); softmax; y = att @ v; out = y @ w_proj

Sharding: 2 heads per core. Each core computes its heads' attention output and
its slice of the output projection (w_proj rows for its heads), producing a
full-shape (T, C) partial; an on-device ReduceScatter sums the partials and
leaves core c with final rows [512c, 512c+512); the host concatenates.

Launch-cost design (the axon PJRT path copies every input buffer to the
device on EVERY execute at ~12 GB/s, so per-launch input bytes dominate):
  - x is uploaded time-sharded: core c holds xT[:, 512c:512c+512) (2 MB);
    an on-device AllGather materializes the full xT (16 MB) in HBM scratch.
    Upload drops 128 MB -> 16 MB across the 8 cores.
  - the output is ReduceScattered on device, so each core's ExternalOutput
    (and its host-supplied zero buffer) is (512, 1024) = 2 MB, not 16 MB.
  - mask tiles, the 128x128 identity, and the ones vectors are generated
    on device with memset/affine_select instead of being uploaded.

Device kernel layout choices (unchanged from the tuned single-output version):
  - q, k are produced transposed ([D*2heads=128, T]) directly by the QKV
    matmul; scores are computed transposed (sT: [s, q]) so that the
    att @ v matmul consumes exp(sT) as the moving operand with no transposes.
  - v is produced transposed and flipped to natural [t, d] layout with PE
    transposes; a ones column is appended per head so the att @ v matmul also
    accumulates the softmax denominator (row 64 of its PSUM output).
  - softmax skips the max-subtraction: scores are ~N(0,1) after the 1/8
    scale, so exp never overflows fp32.
  - matmuls run in float32r (TF32-like, 1 cycle/row on the PE vs 4 for
    fp32); x and the weights travel and feed the QKV GEMM in bf16; the
    output partials and ReduceScatter are bf16.  Measured output RMS
    relative error vs the fp32 reference: 5.2e-3 (gate: 2e-2).
  - the prefix+causal mask is applied multiplicatively to exp(scores) on the
    diagonal blocks only.
"""

import sys

if "/opt/trn_rl_repo" not in sys.path:
    sys.path.insert(0, "/opt/trn_rl_repo")

import numpy as np

import concourse.bacc as bacc
import concourse.mybir as mybir
from concourse.tile import TileContext
from concourse import bass_utils

T = 4096
C = 1024
H = 16
D = 64
NCORES = 8
HPC = H // NCORES          # heads per core = 2
QC = 512                   # q-chunk (moving free dim)
NQ = T // QC               # 8 q-chunks
NCH = C // 128             # 8 contraction chunks for QKV
NT = T // 128              # 32 t-tiles
F32 = mybir.dt.float32
F32R = mybir.dt.float32r
BF16 = mybir.dt.bfloat16
GROUPS = [list(range(NCORES))]

_cache = {}


def _mask_tiles(nf: int):
    """Mask patterns for diagonal score blocks, deduped.

    In sT layout a tile covers s in [128*st, 128*st+128) (partitions) and
    q in [512*j, 512*j+512) (free).  Entry (s, q) is visible iff s <= q or
    s < nf.  A tile needs masking iff st >= 4j (diagonal) and not fully
    visible.  Pattern key: (m, pr) with m = st - 4j, pr = rows fully visible
    from the prefix.  Each pattern is generated on device from (m, pr).
    """
    tiles = {}       # (m, pr) -> index
    patterns = []    # list of (m, pr)
    idx_map = {}     # (j, st) -> index or None (no mask needed)
    p = np.arange(128)[:, None]
    q = np.arange(QC)[None, :]
    for j in range(NQ):
        for st in range(4 * j, 4 * j + 4):
            s0 = 128 * st
            pr = int(np.clip(nf - s0, 0, 128))
            m = st - 4 * j
            causal = (s0 + p) <= (512 * j + q)
            vis = causal | ((s0 + p) < nf)
            if vis.all():
                idx_map[(j, st)] = None
                continue
            key = (m, pr)
            if key not in tiles:
                tiles[key] = len(patterns)
                patterns.append(key)
            idx_map[(j, st)] = tiles[key]
    return patterns, idx_map


def _build(nf: int):
    patterns, idx_map = _mask_tiles(nf)
    n_masks = max(1, len(patterns))

    nc = bacc.Bacc("TRN2", target_bir_lowering=False)

    xs_d = nc.dram_tensor("xs", [C, QC], BF16, kind="ExternalInput")
    wqkv_d = nc.dram_tensor("wqkv", [C, 3 * 128], BF16, kind="ExternalInput")
    wp_d = nc.dram_tensor("wp", [128, C], BF16, kind="ExternalInput")
    y_d = nc.dram_tensor("y_out", [QC, C], BF16, kind="ExternalOutput")

    with TileContext(nc) as tc:
        with tc.tile_pool(name="dram", bufs=1, space="DRAM") as dpool, \
             tc.tile_pool(name="persist", bufs=1) as pp, \
             tc.tile_pool(name="xsb", bufs=2) as xsb, \
             tc.tile_pool(name="agp", bufs=3, space="PSUM") as agp, \
             tc.tile_pool(name="ytp", bufs=2, space="PSUM") as ytp, \
             tc.tile_pool(name="esb", bufs=4) as esb, \
             tc.tile_pool(name="nsb", bufs=2) as nsb, \
             tc.tile_pool(name="pob", bufs=2) as pob:
            # ---- DRAM scratch for collectives (bounce buffers) ----
            xin_b = dpool.tile([C, QC], BF16, tag="xin_b")
            xall_b = dpool.tile([NQ * C, QC], BF16, tag="xall_b")
            yfull_b = dpool.tile([T, C], BF16, tag="yfull_b")
            yrs_b = dpool.tile([QC, C], BF16, tag="yrs_b")

            # x slice -> bounce -> AllGather: xall_b rows [C*j, C*(j+1)) hold
            # xT[:, 512j:512j+512) (core j's upload)
            nc.gpsimd.dma_start(out=xin_b[:, :], in_=xs_d[:, :])
            nc.gpsimd.collective_compute(
                "AllGather", mybir.AluOpType.bypass,
                replica_groups=GROUPS,
                ins=[xin_b.opt()], outs=[xall_b.opt()])

            # per-chunk tensors (separate tiles -> no false WAR deps between
            # later QKV writes and earlier attention reads)
            qTc = [pp.tile([128, QC], F32R, tag=f"qT{i}", name=f"qT{i}") for i in range(NQ)]
            kTc = [pp.tile([128, QC], F32R, tag=f"kT{i}", name=f"kT{i}") for i in range(NQ)]
            # v natural layout per chunk: 4 t-tiles x [v_h0 | ones | v_h1 | ones]
            vsbc = [pp.tile([128, 4, 130], F32R, tag=f"vsb{i}", name=f"vsb{i}") for i in range(NQ)]
            wqkv = pp.tile([128, NCH, 3 * 128], BF16, tag="wqkv")
            wp = pp.tile([128, C], F32R, tag="wp")
            msk = pp.tile([128, n_masks, QC], F32, tag="msk")
            identf = pp.tile([128, 128], F32, tag="identf")
            ident = pp.tile([128, 128], F32R, tag="ident")
            ones2f = pp.tile([1, 256], F32, tag="ones2f")
            ones2 = pp.tile([1, 256], F32R, tag="ones2")
            vone4 = pp.tile([128, 4], F32, tag="vone4")

            # wqkv stays bf16 (PE consumes bf16 directly in the QKV GEMM);
            # wp is converted to f32r since it pairs with the f32r yn operand
            wpb = pp.tile([128, C], BF16, tag="wpb")
            wqkv_r = wqkv_d.ap().rearrange("(n p) m -> p n m", p=128)
            nc.sync.dma_start(out=wqkv[:, :, :], in_=wqkv_r[:, :, :])
            nc.sync.dma_start(out=wpb[:, :], in_=wp_d[:, :])
            nc.vector.tensor_copy(wp[:, :], wpb[:, :])

            # on-device constant generation --------------------------------
            # identity (for PE transposes): 1 where q == p
            nc.gpsimd.memset(identf[:, :], 1.0)
            nc.gpsimd.affine_select(
                identf[:, :], identf[:, :], pattern=[[1, 128]],
                compare_op=mybir.AluOpType.is_equal, fill=0.0,
                base=0, channel_multiplier=-1)
            nc.vector.tensor_copy(ident[:, :], identf[:, :])
            # per-head reciprocal broadcast pattern: cols 0:64 = 1 (head 0
            # rows when sliced at 0:128), cols 192:256 = 1 (head 1 rows when
            # sliced at 128:256), zeros elsewhere
            nc.gpsimd.memset(ones2f[:, :], 1.0)
            nc.gpsimd.affine_select(
                ones2f[:, :], ones2f[:, :], pattern=[[-1, 256]],
                compare_op=mybir.AluOpType.is_ge, fill=0.0,
                base=63, channel_multiplier=0)
            nc.gpsimd.affine_select(
                ones2f[:, :], ones2f[:, :], pattern=[[-1, 256]],
                compare_op=mybir.AluOpType.is_ge, fill=1.0,
                base=191, channel_multiplier=0)
            nc.vector.tensor_copy(ones2[:, :], ones2f[:, :])
            nc.vector.memset(vone4[:, :], 1.0)
            # mask tiles: vis = (q >= p + 128m) | (p < pr)
            for i, (m, pr) in enumerate(patterns):
                nc.gpsimd.memset(msk[:, i, :], 1.0)
                nc.gpsimd.affine_select(
                    msk[:, i, :], msk[:, i, :], pattern=[[1, QC]],
                    compare_op=mybir.AluOpType.is_ge, fill=0.0,
                    base=-128 * m, channel_multiplier=-1)
                if pr > 0:
                    nc.gpsimd.affine_select(
                        msk[:, i, :], msk[:, i, :], pattern=[[0, QC]],
                        compare_op=mybir.AluOpType.is_ge, fill=1.0,
                        base=-pr, channel_multiplier=1)
            # ones columns of the v tiles (softmax denominator accumulators)
            for i in range(NQ):
                nc.vector.tensor_copy(vsbc[i][:, :, 64:65], vone4[:, :])
                nc.vector.tensor_copy(vsbc[i][:, :, 129:130], vone4[:, :])

            def emit_qkv(j):
                xt = xsb.tile([128, NCH, QC], BF16, tag="xt", name=f"xt{j}")
                nc.sync.dma_start(
                    out=xt[:, :, :],
                    in_=xall_b[C * j:C * (j + 1), :].rearrange("(n p) t -> p n t", p=128))
                # v's matmuls first so its DVE copy runs while PE does q/k;
                # the PE-side transposes are deferred until after q/k so the
                # in-order PE stream never waits on that copy
                vstage = xsb.tile([128, QC], F32R, tag="vstage", name=f"vs{j}")
                for m in (2, 0, 1):
                    pm = agp.tile([128, QC], F32, tag="agp", name=f"pm{j}_{m}")
                    for n in range(NCH):
                        nc.tensor.matmul(
                            pm[:, :],
                            wqkv[:, n, 128 * m:128 * (m + 1)],
                            xt[:, n, :],
                            start=(n == 0), stop=(n == NCH - 1),
                        )
                    if m == 0:
                        nc.vector.tensor_copy(qTc[j][:, :], pm[:, :])
                    elif m == 1:
                        nc.vector.tensor_copy(kTc[j][:, :], pm[:, :])
                    else:
                        nc.vector.tensor_copy(vstage[:, :], pm[:, :])
                vtp4 = agp.tile([128, QC], F32, tag="agp", name=f"vtp{j}")
                for k4 in range(4):
                    nc.tensor.transpose(
                        vtp4[:, 128 * k4:128 * (k4 + 1)].bitcast(F32R),
                        vstage[:, 128 * k4:128 * (k4 + 1)],
                        ident[:, :])
                nc.vector.tensor_copy(
                    vsbc[j][:, :, :]
                        .rearrange("p t (h c) -> p t h c", h=2)[:, :, :, 0:64],
                    vtp4[:, :].rearrange("p (t h c) -> p t h c", t=4, h=2))

            def emit_attn(j):
                nst = 4 * j + 4
                yt = [ytp.tile([128, QC], F32, tag="yt", name=f"yt{j}_{h}")
                      for h in range(HPC)]
                # diagonal (masked) groups first so their mask-muls overlap
                # later groups' matmuls instead of sitting on the tail
                glist = list(range(nst // 2))[::-1]
                nb = [0, 0]
                for g in glist:
                    ags, exs = [], []
                    for h in range(HPC):
                        # both heads' score matmuls adjacent in PE order so
                        # the 64-row-packed pairs overlap in the array
                        ag = agp.tile([128, 1024], F32, tag="agp", name=f"ag{j}_{g}_{h}")
                        for u in range(2):
                            st = 2 * g + u
                            nc.tensor.matmul(
                                ag[:, QC * u:QC * (u + 1)],
                                kTc[st // 4][64 * h:64 * h + 64, 128 * (st % 4):128 * (st % 4 + 1)],
                                qTc[j][64 * h:64 * h + 64, :],
                                start=True, stop=True,
                            )
                        ags.append(ag)
                    for h in range(HPC):
                        ex = esb.tile([128, 1024], F32R, tag="ex", name=f"ex{j}_{g}_{h}")
                        nc.scalar.activation(
                            ex[:, :], ags[h][:, :],
                            mybir.ActivationFunctionType.Exp, scale=0.125)
                        for u in range(2):
                            st = 2 * g + u
                            mi = idx_map[(j, st)] if st >= 4 * j else None
                            if mi is not None:
                                eng = nc.vector if u == 0 else nc.gpsimd
                                eng.tensor_mul(
                                    ex[:, QC * u:QC * (u + 1)],
                                    ex[:, QC * u:QC * (u + 1)],
                                    msk[:, mi, :].bitcast(F32R))
                        exs.append(ex)
                    for h in range(HPC):
                        for u in range(2):
                            st = 2 * g + u
                            nc.tensor.matmul(
                                yt[h][0:65, :],
                                vsbc[st // 4][:, st % 4, 65 * h:65 * h + 65],
                                exs[h][:, QC * u:QC * (u + 1)],
                                start=(nb[h] == 0), stop=(nb[h] == nst - 1),
                                skip_group_check=True,
                            )
                            nb[h] += 1
                return yt

            def emit_norm(j, yt):
                rec = [nsb.tile([1, QC], F32R, tag=f"rec{h}", name=f"rec{j}_{h}")
                       for h in range(HPC)]
                with nc.allow_low_precision(reason="f32r holds full-precision reciprocal bits"):
                    for h in range(HPC):
                        nc.vector.reciprocal(rec[h][0:1, :], yt[h][64:65, :])
                rb = agp.tile([128, QC], F32, tag="agp", name=f"rb{j}")
                for h in range(HPC):
                    nc.tensor.matmul(rb[:, :],
                                     ones2[0:1, 128 * h:128 * (h + 1)],
                                     rec[h][0:1, :],
                                     start=(h == 0), stop=(h == HPC - 1),
                                     skip_group_check=True)
                rbs = nsb.tile([128, QC], F32, tag="rbs", name=f"rbs{j}")
                nc.vector.tensor_copy(rbs[:, :], rb[:, :])
                yn = nsb.tile([128, QC], F32R, tag="yn", name=f"yn{j}")
                for h in range(HPC):
                    nc.vector.tensor_mul(
                        yn[64 * h:64 * h + 64, :],
                        yt[h][0:64, :],
                        rbs[64 * h:64 * h + 64, :])
                return yn

            def emit_proj(j, yn):
                q0 = j * QC
                posb = pob.tile([128, 4, 1024], BF16, tag="posb", name=f"posb{j}")
                for k4 in range(4):
                    po = agp.tile([128, 1024], F32, tag="agp", name=f"po{j}_{k4}")
                    for co in range(2):
                        nc.tensor.matmul(
                            po[:, QC * co:QC * (co + 1)],
                            yn[:, 128 * k4:128 * (k4 + 1)],
                            wp[:, QC * co:QC * (co + 1)],
                            start=True, stop=True,
                        )
                    ceng = nc.scalar.copy if k4 % 2 == 0 else nc.vector.tensor_copy
                    ceng(posb[:, k4, :], po[:, :])
                nc.sync.dma_start(
                    out=yfull_b[q0:q0 + QC, :].rearrange("(k p) c -> p k c", p=128),
                    in_=posb[:, :, :])

            # software pipeline: chunk j's normalization + projection are
            # emitted after chunk j+1's QKV, so the in-order PE stream has
            # data-ready QKV matmuls to chew on while the norm chain's
            # reciprocal round-trips through DVE
            prev = None
            for j in range(NQ):
                yn_prev = emit_norm(j - 1, prev) if prev is not None else None
                emit_qkv(j)
                if yn_prev is not None:
                    emit_proj(j - 1, yn_prev)
                prev = emit_attn(j)
            emit_proj(NQ - 1, emit_norm(NQ - 1, prev))

            # on-device all-reduce: core c ends with final rows
            # [512c, 512c+512) = sum over cores of its partial's rows
            nc.gpsimd.collective_compute(
                "ReduceScatter", mybir.AluOpType.add,
                replica_groups=GROUPS,
                ins=[yfull_b.opt()], outs=[yrs_b.opt()])
            nc.gpsimd.dma_start(out=y_d[:, :], in_=yrs_b[:, :])

    nc.compile()
    return nc


class _Runner:
    """Compile once; execute the SPMD NEFF via PJRT shard_map.

    Mirrors bass2jax.run_bass_via_pjrt's multi-core branch, but without
    donating the output buffers so the jitted callable can be re-invoked on
    device-resident inputs (for timing) without re-uploading zeros.
    """

    def __init__(self, nc):
        import jax
        import concourse.mybir as _mybir
        from jax.experimental.shard_map import shard_map
        from jax.sharding import Mesh, PartitionSpec
        from concourse.bass2jax import (_bass_exec_p, install_neuronx_cc_hook,
                                        partition_id_tensor)

        install_neuronx_cc_hook()
        self.nc = nc
        partition_name = nc.partition_id_tensor.name if nc.partition_id_tensor else None
        in_names, out_names, out_avals = [], [], []
        for alloc in nc.m.functions[0].allocations:
            if not isinstance(alloc, _mybir.MemoryLocationSet):
                continue
            name = alloc.memorylocations[0].name
            if alloc.kind == "ExternalInput":
                if name != partition_name:
                    in_names.append(name)
            elif alloc.kind == "ExternalOutput":
                out_names.append(name)
                out_avals.append(jax.core.ShapedArray(
                    tuple(alloc.tensor_shape), _mybir.dt.np(alloc.dtype)))
        self.in_names = list(in_names)
        self.out_names = out_names
        self.out_avals = out_avals
        n_params = len(in_names)
        all_in_names = in_names + out_names
        if partition_name is not None:
            all_in_names.append(partition_name)

        def _body(*args):
            operands = list(args)
            if partition_name is not None:
                operands.append(partition_id_tensor())
            return tuple(_bass_exec_p.bind(
                *operands,
                out_avals=tuple(out_avals),
                in_names=tuple(all_in_names),
                out_names=tuple(out_names),
                lowering_input_output_aliases=(),
                sim_require_finite=True,
                sim_require_nnan=True,
                nc=nc,
            ))

        devices = jax.devices()[:NCORES]
        self.mesh = Mesh(np.asarray(devices), ("core",))
        nin = n_params + len(out_names)
        self.fn = jax.jit(shard_map(
            _body, mesh=self.mesh,
            in_specs=(PartitionSpec("core"),) * nin,
            out_specs=(PartitionSpec("core"),) * len(out_names),
            check_rep=False), keep_unused=True)
        self._zeros = None

    def device_inputs(self, in_maps):
        import jax
        concat = [np.concatenate([np.asarray(m[n]) for m in in_maps], axis=0)
                  for n in self.in_names]
        if self._zeros is None:
            self._zeros = [
                jax.device_put(np.zeros((NCORES * a.shape[0], *a.shape[1:]), a.dtype))
                for a in self.out_avals]
        return [jax.device_put(c) for c in concat] + self._zeros

    def run(self, dev_inputs):
        outs = self.fn(*dev_inputs)
        return outs

    def gather(self, outs):
        res = []
        for c in range(NCORES):
            res.append({
                name: np.asarray(outs[i]).reshape(NCORES, *self.out_avals[i].shape)[c]
                for i, name in enumerate(self.out_names)})
        return res


def get_runner(num_frames=64):
    nf = int(np.asarray(num_frames))
    if nf not in _cache:
        _cache[nf] = _Runner(_build(nf))
    return _cache[nf]


def make_in_maps(x, w_attn, w_proj):
    bf16 = mybir.dt.np(BF16)
    xT = np.ascontiguousarray(x.T)
    in_maps = []
    for c in range(NCORES):
        h0, h1 = HPC * c, HPC * c + 1
        wq = np.concatenate([w_attn[:, D * h0:D * h0 + D],
                             w_attn[:, D * h1:D * h1 + D]], axis=1)
        wk = np.concatenate([w_attn[:, C + D * h0:C + D * h0 + D],
                             w_attn[:, C + D * h1:C + D * h1 + D]], axis=1)
        wv = np.concatenate([w_attn[:, 2 * C + D * h0:2 * C + D * h0 + D],
                             w_attn[:, 2 * C + D * h1:2 * C + D * h1 + D]], axis=1)
        wqkv = np.ascontiguousarray(np.concatenate([wq, wk, wv], axis=1))
        wp = np.ascontiguousarray(
            np.concatenate([w_proj[D * h0:D * h0 + D, :],
                            w_proj[D * h1:D * h1 + D, :]], axis=0))
        in_maps.append({
            "xs": np.ascontiguousarray(xT[:, QC * c:QC * (c + 1)]).astype(bf16),
            "wqkv": wqkv.astype(bf16), "wp": wp.astype(bf16),
        })
    return in_maps


def kernel(x, w_attn, w_proj, num_frames):
    x = np.asarray(x, dtype=np.float32)
    w_attn = np.asarray(w_attn, dtype=np.float32)
    w_proj = np.asarray(w_proj, dtype=np.float32)

    runner = get_runner(num_frames)
    in_maps = make_in_maps(x, w_attn, w_proj)
    import jax, time
    try:
        outs = runner.run(runner.device_inputs(in_maps))
        jax.block_until_ready(outs)
    except Exception:
        # a wedged NeuronCore recovers after the terminal recycles (~90 s)
        time.sleep(100)
        runner._zeros = None
        outs = runner.run(runner.device_inputs(in_maps))
        jax.block_until_ready(outs)
    results = runner.gather(outs)
    return np.concatenate([r["y_out"] for r in results], axis=0).astype(np.float32)



# revision 13
# speedup vs baseline: 2.4068x; 2.4068x over previous
"""Causal self-attention with dense global prefix on ONE Trainium2 NeuronCore.

Reference computation (T=4096, C=1024, H=16, D=64):
    qkv = x @ w_attn; q,k,v per head; scores = q k^T / sqrt(D)
    mask = causal | (col < num_frames); softmax; y = att @ v; out = y @ w_proj

Why single-core: the axon PJRT per-execute dispatch floor scales with device
count (~5.5 ms for an 8-core shard_map launch, ~0.5 ms for a 1-core fast-path
launch, measured with a no-op NEFF), while the whole problem is only ~36.5
G MACs (~1.4 ms of PE) + 151M exps (~1.2 ms of ACT, the only exp engine).
The 8-core tensor-parallel version pays 5 ms of launch tax to save ~1 ms of
compute; one core wins by ~3x.

Device kernel layout (evolved from the 8-core 2-head/core version; same
per-head-pair inner structure, now looped over 8 head-pair groups):
  - q, k produced transposed ([128 = 2 heads x 64, T]) by the QKV matmul;
    scores computed transposed (sT: [s, q]) so att @ v consumes exp(sT) as
    the moving operand with no transposes.
  - k^T and v (natural layout, with a ones column per head appended so the
    att @ v matmul also accumulates the softmax denominator in row 64) are
    bounced through HBM scratch and re-streamed per (chunk, head-pair):
    37.7 MB each per launch, ~0.2 ms of DMA hidden under ~1.4 ms of compute.
    SBUF can't hold K+V+weights for all 16 heads at once.
  - softmax skips the max-subtraction: scores are ~N(0,1) after the 1/8
    scale, so exp never overflows.  Denominator reciprocals use the fast
    custom-DVE approx (18 correct bits) and are broadcast across partitions
    with the ones-vector PE matmul trick.
  - all matmul operands are bf16 except the norm/broadcast path (f32r);
    PSUM accumulation is fp32 throughout.  Measured output RMS relative
    error vs the fp32 reference: ~7e-3 (gate: 2e-2).
  - per-launch host I/O: xT (8 MiB), w_attn (6), w_proj (2) in, y (8) out,
    all bf16 -- but payload bytes are nearly free on this tunnel; the
    binding constraint is the per-execute dispatch, minimized via jax's
    C++ fast-path dispatch (fast_dispatch_compile).
"""

import sys

if "/opt/trn_rl_repo" not in sys.path:
    sys.path.insert(0, "/opt/trn_rl_repo")

import numpy as np

import concourse.bacc as bacc
import concourse.mybir as mybir
from concourse.tile import TileContext

T = 4096
C = 1024
H = 16
D = 64
QC = 512                   # q-chunk (moving free dim)
NQ = T // QC               # 8 q-chunks
NCH = C // 128             # 8 contraction chunks / feature chunks
NHG = H // 2               # 8 head-pair groups
NT = T // 128              # 32 t-tiles
F32 = mybir.dt.float32
F32R = mybir.dt.float32r
BF16 = mybir.dt.bfloat16

_cache = {}


def _mask_tiles(nf: int):
    """Mask patterns for diagonal score blocks, deduped.

    In sT layout a tile covers s in [128*st, 128*st+128) (partitions) and
    q in [512*j, 512*j+512) (free).  Entry (s, q) is visible iff s <= q or
    s < nf.  A tile needs masking iff st >= 4j (diagonal) and not fully
    visible.  Pattern key: (m, pr) with m = st - 4j, pr = rows fully visible
    from the prefix.  Each pattern is generated on device from (m, pr).
    """
    tiles = {}
    patterns = []
    idx_map = {}
    p = np.arange(128)[:, None]
    q = np.arange(QC)[None, :]
    for j in range(NQ):
        for st in range(4 * j, 4 * j + 4):
            s0 = 128 * st
            pr = int(np.clip(nf - s0, 0, 128))
            m = st - 4 * j
            causal = (s0 + p) <= (512 * j + q)
            vis = causal | ((s0 + p) < nf)
            if vis.all():
                idx_map[(j, st)] = None
                continue
            key = (m, pr)
            if key not in tiles:
                tiles[key] = len(patterns)
                patterns.append(key)
            idx_map[(j, st)] = tiles[key]
    return patterns, idx_map


def _build(nf: int):
    patterns, idx_map = _mask_tiles(nf)
    n_masks = max(1, len(patterns))

    nc = bacc.Bacc("TRN2", target_bir_lowering=False)

    xs_d = nc.dram_tensor("xs", [C, T], BF16, kind="ExternalInput")
    wqkv_d = nc.dram_tensor("wqkv", [C, 3 * C], BF16, kind="ExternalInput")
    wp_d = nc.dram_tensor("wp", [C, C], BF16, kind="ExternalInput")
    y_d = nc.dram_tensor("y_out", [T, C], BF16, kind="ExternalOutput")

    with TileContext(nc) as tc:
        with tc.tile_pool(name="dram", bufs=1, space="DRAM") as dpool, \
             tc.tile_pool(name="persist", bufs=1) as pp, \
             tc.tile_pool(name="xsb", bufs=2) as xsb, \
             tc.tile_pool(name="stg", bufs=1) as stg, \
             tc.tile_pool(name="qp", bufs=2) as qp, \
             tc.tile_pool(name="kvp", bufs=2) as kvp, \
             tc.tile_pool(name="agp", bufs=2, space="PSUM") as agp, \
             tc.tile_pool(name="ytp", bufs=4, space="PSUM") as ytp, \
             tc.tile_pool(name="esb", bufs=2) as esb, \
             tc.tile_pool(name="vsg", bufs=3) as vsg, \
             tc.tile_pool(name="nsb", bufs=1) as nsb, \
             tc.tile_pool(name="ynp", bufs=2) as ynp, \
             tc.tile_pool(name="pob", bufs=1) as pob:
            # ---- HBM scratch for K/V bounce ----
            # kT_d[n, p, t]: feature chunk n = heads 2n,2n+1; partition p =
            # feature within chunk; t = key position.
            kT_d = dpool.tile([NCH, 128, T], BF16, tag="kT_d")
            # v_d[tt, p, h, f]: t-tile tt, t-within-tile p, head h, feature f
            # (f=64 is the appended ones column).
            v_d = dpool.tile([NT, 128, H, 65], BF16, tag="v_d")

            wqkv = pp.tile([128, NCH, 3 * C], BF16, tag="wqkv")
            wp = pp.tile([128, NCH, C], BF16, tag="wp")
            mskf = pp.tile([128, QC], F32, tag="mskf")
            msk = pp.tile([128, n_masks, QC], BF16, tag="msk")
            identf = pp.tile([128, 128], F32, tag="identf")
            ident = pp.tile([128, 128], F32R, tag="ident")
            ones2f = pp.tile([1, 256], F32, tag="ones2f")
            ones2 = pp.tile([1, 256], F32R, tag="ones2")

            wqkv_r = wqkv_d.ap().rearrange("(n p) m -> p n m", p=128)
            nc.sync.dma_start(out=wqkv[:, :, :], in_=wqkv_r[:, :, :])
            wp_r = wp_d.ap().rearrange("(n p) c -> p n c", p=128)
            nc.sync.dma_start(out=wp[:, :, :], in_=wp_r[:, :, :])

            # on-device constant generation --------------------------------
            nc.gpsimd.memset(identf[:, :], 1.0)
            nc.gpsimd.affine_select(
                identf[:, :], identf[:, :], pattern=[[1, 128]],
                compare_op=mybir.AluOpType.is_equal, fill=0.0,
                base=0, channel_multiplier=-1)
            nc.vector.tensor_copy(ident[:, :], identf[:, :])
            # per-head reciprocal broadcast pattern: cols 0:64 = 1 (head 0
            # rows when sliced at 0:128), cols 192:256 = 1 (head 1 rows when
            # sliced at 128:256)
            nc.gpsimd.memset(ones2f[:, :], 1.0)
            nc.gpsimd.affine_select(
                ones2f[:, :], ones2f[:, :], pattern=[[-1, 256]],
                compare_op=mybir.AluOpType.is_ge, fill=0.0,
                base=63, channel_multiplier=0)
            nc.gpsimd.affine_select(
                ones2f[:, :], ones2f[:, :], pattern=[[-1, 256]],
                compare_op=mybir.AluOpType.is_ge, fill=1.0,
                base=191, channel_multiplier=0)
            nc.vector.tensor_copy(ones2[:, :], ones2f[:, :])
            # mask tiles: vis = (q >= p + 128m) | (p < pr)
            for i, (m, pr) in enumerate(patterns):
                nc.gpsimd.memset(mskf[:, :], 1.0)
                nc.gpsimd.affine_select(
                    mskf[:, :], mskf[:, :], pattern=[[1, QC]],
                    compare_op=mybir.AluOpType.is_ge, fill=0.0,
                    base=-128 * m, channel_multiplier=-1)
                if pr > 0:
                    nc.gpsimd.affine_select(
                        mskf[:, :], mskf[:, :], pattern=[[0, QC]],
                        compare_op=mybir.AluOpType.is_ge, fill=1.0,
                        base=-pr, channel_multiplier=1)
                nc.gpsimd.tensor_copy(msk[:, i, :], mskf[:, :])

            # v staging: [t-within-tile, t-tile-of-chunk, head, 65]; the
            # ones column (f=64) is written once and survives because the
            # per-chunk transpose copies only touch f=0:64.
            vstg = stg.tile([128, 4, H, 65], BF16, tag="vstg")
            nc.vector.memset(vstg[:, :, :, 64:65], 1.0)

            def emit_qkv(j):
                xt = xsb.tile([128, NCH, QC], BF16, tag="xt", name=f"xt{j}")
                nc.sync.dma_start(
                    out=xt[:, :, :],
                    in_=xs_d[:, QC * j:QC * (j + 1)].rearrange(
                        "(n p) t -> p n t", p=128))
                qsb = qp.tile([128, NCH, QC], BF16, tag="qsb", name=f"q{j}")
                kstg = stg.tile([128, NCH, QC], BF16, tag="kstg",
                                name=f"k{j}")
                vpend = []

                def flush_v(count):
                    # PE-transpose pending v chunks to natural layout
                    while len(vpend) > count:
                        m, vs = vpend.pop(0)
                        vtp4 = agp.tile([128, QC], F32, tag="agp",
                                        name=f"vtp{j}_{m}")
                        for k4 in range(4):
                            nc.tensor.transpose(
                                vtp4[:, 128 * k4:128 * (k4 + 1)]
                                    .bitcast(F32R),
                                vs[:, 128 * k4:128 * (k4 + 1)],
                                ident[:, :])
                        nc.vector.tensor_copy(
                            vstg[:, :, 2 * m:2 * m + 2, 0:64],
                            vtp4[:, :].rearrange("p (t h c) -> p t h c",
                                                 t=4, h=2))

                # v first so its transposes interleave with later matmuls;
                # k next so its DMA-out starts before attention needs it.
                for sec in (2, 1, 0):
                    for m in range(NCH):
                        pm = agp.tile([128, QC], F32, tag="agp",
                                      name=f"pm{j}_{sec}_{m}")
                        for n in range(NCH):
                            nc.tensor.matmul(
                                pm[:, :],
                                wqkv[:, n, C * sec + 128 * m:
                                     C * sec + 128 * (m + 1)],
                                xt[:, n, :],
                                start=(n == 0), stop=(n == NCH - 1),
                            )
                        if sec == 0:
                            nc.vector.tensor_copy(qsb[:, m, :], pm[:, :])
                        elif sec == 1:
                            nc.vector.tensor_copy(kstg[:, m, :], pm[:, :])
                        else:
                            vs = vsg.tile([128, QC], F32R, tag="vs",
                                          name=f"vs{j}_{m}")
                            nc.vector.tensor_copy(vs[:, :], pm[:, :])
                            vpend.append((m, vs))
                            flush_v(2)
                    if sec == 2:
                        flush_v(0)
                        nc.sync.dma_start(
                            out=v_d[4 * j:4 * (j + 1), :, :, :].rearrange(
                                "t p h f -> p t h f"),
                            in_=vstg[:, :, :, :])
                    if sec == 1:
                        nc.sync.dma_start(
                            out=kT_d[:, :, QC * j:QC * (j + 1)].rearrange(
                                "n p t -> p n t"),
                            in_=kstg[:, :, :])
                return qsb

            def emit_norm(j, hg, yt, yn):
                dn = nsb.tile([1, 1024], F32, tag="dn",
                              name=f"dn{j}_{hg}")
                rec = nsb.tile([1, 1024], F32, tag="rec",
                               name=f"rec{j}_{hg}")
                recr = nsb.tile([1, 1024], F32R, tag="recr",
                                name=f"recr{j}_{hg}")
                # custom-DVE ops are SBUF-only on HW; bounce the PSUM
                # denominator rows through SBUF first
                for h in range(2):
                    nc.vector.tensor_copy(dn[0:1, QC * h:QC * (h + 1)],
                                          yt[h][64:65, :])
                with nc.allow_low_precision(
                        reason="18-bit reciprocal of softmax denominator"):
                    nc.vector.reciprocal_approx_fast(rec[0:1, :], dn[0:1, :])
                nc.vector.tensor_copy(recr[0:1, :], rec[0:1, :])
                rb = agp.tile([128, QC], F32, tag="agp",
                              name=f"rb{j}_{hg}")
                for h in range(2):
                    nc.tensor.matmul(
                        rb[:, :],
                        ones2[0:1, 128 * h:128 * (h + 1)],
                        recr[0:1, QC * h:QC * (h + 1)],
                        start=(h == 0), stop=(h == 1),
                        skip_group_check=True)
                rbs = nsb.tile([128, QC], F32, tag="rbs",
                               name=f"rbs{j}_{hg}")
                nc.vector.tensor_copy(rbs[:, :], rb[:, :])
                for h in range(2):
                    nc.vector.tensor_mul(
                        yn[64 * h:64 * h + 64, hg, :],
                        yt[h][0:64, :],
                        rbs[64 * h:64 * h + 64, :])

            def emit_attn(j, qsb):
                nst = 4 * j + 4
                yn = ynp.tile([128, NHG, QC], BF16, tag="yn", name=f"yn{j}")
                pending = None
                for hg in range(NHG):
                    ksb = kvp.tile([128, nst, 128], BF16, tag="ksb",
                                   name=f"ksb{j}_{hg}")
                    nc.sync.dma_start(
                        out=ksb[:, :, :],
                        in_=kT_d[hg, :, 0:128 * nst].rearrange(
                            "p (s t) -> p s t", t=128))
                    vsb = kvp.tile([128, nst, 2, 65], BF16, tag="vsb",
                                   name=f"vsb{j}_{hg}")
                    nc.sync.dma_start(
                        out=vsb[:, :, :, :],
                        in_=v_d[0:nst, :, 2 * hg:2 * hg + 2, :].rearrange(
                            "s p h f -> p s h f"))
                    yt = [ytp.tile([128, QC], F32, tag="yt",
                                   name=f"yt{j}_{hg}_{h}")
                          for h in range(2)]
                    nb = [0, 0]
                    # diagonal (masked) groups first so their mask-muls
                    # overlap later groups' matmuls
                    glist = list(range(nst // 2))[::-1]
                    for gi, g in enumerate(glist):
                        ags, exs = [], []
                        for h in range(2):
                            ag = agp.tile([128, 1024], F32, tag="agp",
                                          name=f"ag{j}_{hg}_{g}_{h}")
                            for u in range(2):
                                st = 2 * g + u
                                nc.tensor.matmul(
                                    ag[:, QC * u:QC * (u + 1)],
                                    ksb[64 * h:64 * h + 64, st, :],
                                    qsb[64 * h:64 * h + 64, hg, :],
                                    start=True, stop=True,
                                )
                            ags.append(ag)
                        for h in range(2):
                            ex = esb.tile([128, 1024], BF16, tag="ex",
                                          name=f"ex{j}_{hg}_{g}_{h}")
                            nc.scalar.activation(
                                ex[:, :], ags[h][:, :],
                                mybir.ActivationFunctionType.Exp,
                                scale=0.125)
                            for u in range(2):
                                st = 2 * g + u
                                mi = idx_map[(j, st)] if st >= 4 * j else None
                                if mi is not None:
                                    eng = nc.vector if u == 0 else nc.gpsimd
                                    eng.tensor_mul(
                                        ex[:, QC * u:QC * (u + 1)],
                                        ex[:, QC * u:QC * (u + 1)],
                                        msk[:, mi, :])
                            exs.append(ex)
                        for h in range(2):
                            for u in range(2):
                                st = 2 * g + u
                                nc.tensor.matmul(
                                    yt[h][0:65, :],
                                    vsb[:, st, h, :],
                                    exs[h][:, QC * u:QC * (u + 1)],
                                    start=(nb[h] == 0),
                                    stop=(nb[h] == nst - 1),
                                    skip_group_check=True,
                                )
                                nb[h] += 1
                        if gi == 0 and pending is not None:
                            # previous head-pair's normalization, emitted
                            # after this pair's first score matmuls so the
                            # PE never waits on the DVE reciprocal
                            emit_norm(j, pending[0], pending[1], yn)
                            pending = None
                    pending = (hg, yt)
                emit_norm(j, pending[0], pending[1], yn)
                return yn

            def emit_proj(j, yn):
                q0 = j * QC
                posb = pob.tile([128, 4, C], BF16, tag="posb",
                                name=f"posb{j}")
                for s in range(4):
                    po = agp.tile([128, C], F32, tag="agp",
                                  name=f"po{j}_{s}")
                    for f in range(NCH):
                        for co in range(2):
                            nc.tensor.matmul(
                                po[:, QC * co:QC * (co + 1)],
                                yn[:, f, 128 * s:128 * (s + 1)],
                                wp[:, f, QC * co:QC * (co + 1)],
                                start=(f == 0), stop=(f == NCH - 1),
                                skip_group_check=True,
                            )
                    nc.vector.tensor_copy(posb[:, s, :], po[:, :])
                nc.sync.dma_start(
                    out=y_d[q0:q0 + QC, :].rearrange("(k p) c -> p k c",
                                                     p=128),
                    in_=posb[:, :, :])

            # software pipeline: chunk j's projection is emitted after
            # chunk j+1's QKV so the PE always has data-ready matmuls
            prev_yn = None
            for j in range(NQ):
                qsb = emit_qkv(j)
                if prev_yn is not None:
                    emit_proj(j - 1, prev_yn)
                prev_yn = emit_attn(j, qsb)
            emit_proj(NQ - 1, prev_yn)

    nc.compile()
    return nc


class _Runner:
    """Compile once; execute the single-core NEFF via PJRT fast dispatch.

    Mirrors bass2jax.run_bass_via_pjrt's single-core branch, but without
    donating the output buffers (so the callable can be re-invoked on
    device-resident inputs for timing) and with jax's C++ fast-path
    dispatch (saves ~0.5 ms/launch of effects-path overhead).
    """

    def __init__(self, nc):
        import jax
        import concourse.mybir as _mybir
        from concourse.bass2jax import (_bass_exec_p, install_neuronx_cc_hook,
                                        partition_id_tensor,
                                        fast_dispatch_compile)

        install_neuronx_cc_hook()
        self.nc = nc
        partition_name = nc.partition_id_tensor.name if nc.partition_id_tensor else None
        in_names, in_avals, out_names, out_avals = [], [], [], []
        for alloc in nc.m.functions[0].allocations:
            if not isinstance(alloc, _mybir.MemoryLocationSet):
                continue
            name = alloc.memorylocations[0].name
            if alloc.kind == "ExternalInput":
                if name != partition_name:
                    in_names.append(name)
                    in_avals.append(jax.core.ShapedArray(
                        tuple(alloc.tensor_shape), _mybir.dt.np(alloc.dtype)))
            elif alloc.kind == "ExternalOutput":
                out_names.append(name)
                out_avals.append(jax.core.ShapedArray(
                    tuple(alloc.tensor_shape), _mybir.dt.np(alloc.dtype)))
        self.in_names = list(in_names)
        self.out_names = out_names
        self.out_avals = out_avals
        all_in_names = in_names + out_names
        if partition_name is not None:
            all_in_names.append(partition_name)

        def _body(*args):
            operands = list(args)
            if partition_name is not None:
                operands.append(partition_id_tensor())
            return tuple(_bass_exec_p.bind(
                *operands,
                out_avals=tuple(out_avals),
                in_names=tuple(all_in_names),
                out_names=tuple(out_names),
                lowering_input_output_aliases=(),
                sim_require_finite=True,
                sim_require_nnan=True,
                nc=nc,
            ))

        avals = [jax.ShapeDtypeStruct(a.shape, a.dtype)
                 for a in (in_avals + out_avals)]
        try:
            self.fn = fast_dispatch_compile(
                lambda: jax.jit(_body, keep_unused=True).lower(
                    *avals).compile())
        except Exception:
            self.fn = jax.jit(_body, keep_unused=True)
        self._zeros = None

    def device_inputs(self, in_maps):
        import jax
        (in_map,) = in_maps
        if self._zeros is None:
            self._zeros = [jax.device_put(np.zeros(a.shape, a.dtype))
                           for a in self.out_avals]
        return [jax.device_put(np.asarray(in_map[n])) for n in self.in_names] \
            + self._zeros

    def run(self, dev_inputs):
        return self.fn(*dev_inputs)

    def gather(self, outs):
        return [{name: np.asarray(outs[i])
                 for i, name in enumerate(self.out_names)}]


def get_runner(num_frames=64):
    nf = int(np.asarray(num_frames))
    if nf not in _cache:
        _cache[nf] = _Runner(_build(nf))
    return _cache[nf]


def make_in_maps(x, w_attn, w_proj):
    bf16 = mybir.dt.np(BF16)
    xT = np.ascontiguousarray(x.T).astype(bf16)
    return [{
        "xs": xT,
        "wqkv": np.ascontiguousarray(w_attn).astype(bf16),
        "wp": np.ascontiguousarray(w_proj).astype(bf16),
    }]


def kernel(x, w_attn, w_proj, num_frames):
    x = np.asarray(x, dtype=np.float32)
    w_attn = np.asarray(w_attn, dtype=np.float32)
    w_proj = np.asarray(w_proj, dtype=np.float32)

    runner = get_runner(num_frames)
    in_maps = make_in_maps(x, w_attn, w_proj)
    import jax, time
    try:
        outs = runner.run(runner.device_inputs(in_maps))
        jax.block_until_ready(outs)
    except Exception:
        # a wedged NeuronCore recovers after the terminal recycles (~90 s)
        time.sleep(100)
        runner._zeros = None
        outs = runner.run(runner.device_inputs(in_maps))
        jax.block_until_ready(outs)
    results = runner.gather(outs)
    return results[0]["y_out"].astype(np.float32)
